# revision 7
# baseline (speedup 1.0000x reference)
"""Multi-head masked self-attention on 8 TRN2 NeuronCores.

Problem: B=4, S=2048, D=1024, H=16 heads (hd=64), fp32.
  q,k,v = x@W* + b*; causal softmax(q k^T / 8) @ v; out = ctx @ Wo + bo.

Sharding: core c -> (batch b = c//2, head-group g = c%2 of 8 heads).
Each core computes a partial output projection over its 512 hidden dims;
the host sums the two partials per batch and adds bo (with bv@Wo folded in,
so no v-bias work on device).

On-device layout strategy (no on-device transposes needed):
  - host passes xT = x[b].T  [D, S] in bf16; all GEMM-side operands
    (x, Wq/Wk/Wv/Wo, q^T, k^T, ctx^T) are bf16: same PE throughput as
    f32r (1 col/cycle) but fast-weight-load halves LDWEIGHTS, and input
    DMA bytes halve
  - q^T, k^T computed directly as [512, S] (lhsT = W chunk, rhs = xT chunk)
  - v computed in natural [S, 512] layout, stored with a ones-column per
    head ("v_aug", f32r [S, 8*65]) so the ctx matmul accumulates the
    softmax denominator in PSUM row 64 for free
  - scores are computed transposed: st[sk, sq] = k q^T.  The two heads of
    a pair (2i, 2i+1) live in partitions 0:64 / 64:128 of the same kT/qT
    chunk, so their K=64 score matmuls land on PE row-groups (0,0) and
    (64,0) and run CONCURRENTLY (row tiling) when emitted back-to-back —
    2x on the score matmuls vs serial heads
  - exp via ACT (softmax without max-subtraction: scores are O(+-10));
    optionally a fraction of off-diagonal blocks run a one-instruction
    Schraudolph exp on DVE (i32 = A*s + B, bitcast to f32) to offload ACT
  - causal masking by multiplying only the triangular 128-col band of
    diagonal blocks with precomputed 0/1 masks (not the full block)
  - ctx^T[hd, sq] accumulated in PSUM = v_aug^T.T @ exp; normalization:
    fast approximate reciprocal of the den row straight out of PSUM,
    gpsimd partition_broadcast, multiply on DVE during evacuation
  - output projection uses ctx^T directly as lhsT (again no transpose);
    ctx^T aliases qT's storage (each qT j-tile dies as its wave completes)
  - QKV projections, attention waves, and the output projection are
    emitted so the Tile scheduler keeps the PE dense and warm
"""

import numpy as np
import ml_dtypes

import concourse.bass as bass
import concourse.mybir as mybir
import concourse.tile as tile
from concourse import bacc
from concourse.bass import ts
from concourse.bass_utils import run_bass_kernel_spmd

F32 = mybir.dt.float32
F32R = mybir.dt.float32r
BF16 = mybir.dt.bfloat16
I32 = mybir.dt.int32
I16 = mybir.dt.int16
AF = mybir.ActivationFunctionType
ALU = mybir.AluOpType

B, S, D, H, HD = 4, 2048, 1024, 16, 64
G = 2                 # head groups (cores per batch)
DH = D // G           # hidden dims per core = 512
HPC = H // G          # heads per core = 8
NHP = HPC // 2        # head pairs per core = 4
NCORES = 8

NSQ = S // 512        # 4 sq tiles of 512
NSK = S // 128        # 16 sk chunks of 128
NFC = D // 128        # 8 feature chunks
NOC = DH // 128       # 4 out-dim chunks of the per-core hidden

NWARM = 48            # HAM warmup matmuls while input DMAs land

# Schraudolph exp in bf16: exp(0.125*s) ~= bitcast_bf16(round(SCH_A*s + SCH_B)
# as int16); max rel err ~3.3%, mean +1.0% (fitted for s ~ N(0, 64))
SCH_A = float(np.float32(0.125 * (2 ** 7) / np.log(2)))
SCH_B = float(np.float32(127 * 128 - 5.5))

# per-diagonal-block (t = blk - 4j) mask ranges: (w-relative col range of
# the mask multiply, absolute col range of the mask source)
MASK_APPLY = {
    0: ((0, 128), (0, 128)),
    1: ((0, 256), (0, 256)),
    2: ((0, 128), (256, 384)),
    3: ((0, 256), (256, 512)),
}


def build_program(dve_num=0, dve_den=2):
    """Build the single-core SPMD Bass program (same program on all 8 cores).

    Off-diagonal exp blocks with (counter % dve_den) < dve_num run on DVE
    via Schraudolph; the rest (and all diagonal blocks) run exact exp on ACT.
    """
    nc = bacc.Bacc("TRN2", target_bir_lowering=False, debug=False)

    xT_d = nc.dram_tensor("xT", [D, S], BF16, kind="ExternalInput").ap()
    wq_d = nc.dram_tensor("wq", [D, DH], BF16, kind="ExternalInput").ap()
    wk_d = nc.dram_tensor("wk", [D, DH], BF16, kind="ExternalInput").ap()
    wv_d = nc.dram_tensor("wv", [D, DH], BF16, kind="ExternalInput").ap()
    wo_d = nc.dram_tensor("wo", [DH, D], BF16, kind="ExternalInput").ap()
    bqt_d = nc.dram_tensor("bqt", [128, NOC], F32, kind="ExternalInput").ap()
    bkt_d = nc.dram_tensor("bkt", [128, NOC], F32, kind="ExternalInput").ap()
    mask_d = nc.dram_tensor("masks", [128, 4, 2, 512], BF16,
                            kind="ExternalInput").ap()
    po_d = nc.dram_tensor("po", [S, D], F32, kind="ExternalOutput").ap()

    with tile.TileContext(nc) as tc:
        _emit(tc, xT_d, wq_d, wk_d, wv_d, wo_d, bqt_d, bkt_d, mask_d, po_d,
              dve_num, dve_den)
    nc.compile()
    return nc


def _emit(tc, xT_d, wq_d, wk_d, wv_d, wo_d, bqt_d, bkt_d, mask_d, po_d,
          dve_num, dve_den):
    nc = tc.nc
    PS = bass.MemorySpace.PSUM

    with (
        tc.tile_pool(name="persist", bufs=1) as persist,
        tc.tile_pool(name="qkv", bufs=1) as qkv_pool,
        tc.tile_pool(name="exp", bufs=6) as exp_pool,
        tc.tile_pool(name="small", bufs=2) as small_pool,
        tc.tile_pool(name="ps_mm", bufs=3, space=PS) as ps_mm,
        tc.tile_pool(name="ps_ctx", bufs=2, space=PS) as ps_ctx,
    ):
        bqt = persist.tile([128, NOC], F32)
        bkt = persist.tile([128, NOC], F32)
        nc.sync.dma_start(bqt[:], bqt_d[:])
        nc.sync.dma_start(bkt[:], bkt_d[:])

        # HAM pre-warm: throwaway matmuls on zeros while input DMAs land,
        # so the PE clock is at 2.4 GHz when real work starts
        zw = persist.tile([128, 512], BF16)
        nc.vector.memset(zw[:], 0.0)
        pwarm = ps_mm.tile([128, 2, 512], F32, name="mm")
        for i in range(NWARM):
            nc.tensor.matmul(pwarm[:, i % 2, :], zw[:, 0:128], zw[:],
                             start=True, stop=True)

        # masks: [128, t, dup, 512]; dup dim matches e's head-pair dim
        masks = persist.tile([128, 4, 2, 512], BF16)
        nc.sync.dma_start(masks[:], mask_d[:])

        # persistent activations.  qT doubles as ctx^T storage: wave j's
        # evacuation overwrites qT[:, :, j-tile] right after the last
        # score matmul that reads it (disjoint partition rows per head).
        qT = qkv_pool.tile([128, NOC, S], BF16)       # q^T + bq, then ctx^T
        kT = qkv_pool.tile([128, NOC, S], BF16)       # k^T + bk   [512, S]
        vA = qkv_pool.tile([128, NSK, HPC, HD + 1], BF16)  # v + ones col
        nc.vector.memset(vA[:, :, :, HD:HD + 1], 1.0)

        exp_ctr = [0]

        with (
            tc.tile_pool(name="wtsA", bufs=1) as wtsA,
            tc.tile_pool(name="xin", bufs=2) as xin,
            tc.tile_pool(name="woC", bufs=1) as woC,
            tc.tile_pool(name="poC", bufs=3) as poC,
        ):
            xts = {}
            xts[0] = xin.tile([128, NFC, 512], BF16, name="xt")
            wq = wtsA.tile([128, NFC, DH], BF16)
            wk = wtsA.tile([128, NFC, DH], BF16)
            wv = wtsA.tile([128, NFC, DH], BF16)
            wo = woC.tile([128, NOC, D], BF16)
            # interleave x/wq chunks so the first K-chain starts ASAP
            for fc in range(NFC):
                nc.sync.dma_start(xts[0][:, fc], xT_d[ts(fc, 128), ts(0, 512)])
                nc.sync.dma_start(wq[:, fc], wq_d[ts(fc, 128), :])
            for fc in range(NFC):
                nc.sync.dma_start(wk[:, fc], wk_d[ts(fc, 128), :])
            for fc in range(NFC):
                nc.sync.dma_start(wv[:, fc], wv_d[ts(fc, 128), :])

            # --- attention wave pieces (emitted per unit so fill work can
            # be woven between units) -----------------------------------
            ctxps = {}

            def emit_blk(hp, j, blk):
                """One (head-pair, sq-tile, k-block) unit: the two heads'
                K=64 score matmuls run concurrently on PE row-groups 0/64."""
                nblk = 4 * j + 4
                hA, hB = 2 * hp, 2 * hp + 1
                if blk == 0:
                    ctxps[hp] = (ps_ctx.tile([65, 512], F32, name="ctxp"),
                                 ps_ctx.tile([65, 512], F32, name="ctxp"))
                ctxA, ctxB = ctxps[hp]
                t = blk - 4 * j  # >= 0 on diagonal blocks
                hi = t >= 2
                w = 256 if hi else 512
                cq = 256 if hi else 0
                stp = ps_mm.tile([128, 2, 512], F32, name="mm")
                nc.tensor.matmul(stp[:, 0, 0:w], kT[0:64, hp, ts(blk, 128)],
                                 qT[0:64, hp, ts(j, 512)][:, cq:512],
                                 start=True, stop=True)
                nc.tensor.matmul(stp[:, 1, 0:w], kT[64:128, hp, ts(blk, 128)],
                                 qT[64:128, hp, ts(j, 512)][:, cq:512],
                                 start=True, stop=True)
                e = exp_pool.tile([128, 2, 512], BF16, name="e")
                use_dve = (t < 0 and (exp_ctr[0] % dve_den) < dve_num)
                if t < 0:
                    exp_ctr[0] += 1
                if use_dve:
                    nc.vector.tensor_scalar(
                        e[:, :, 0:w].bitcast(I16), stp[:, :, 0:w],
                        SCH_A, SCH_B, ALU.mult, ALU.add)
                else:
                    nc.scalar.activation(e[:, :, 0:w], stp[:, :, 0:w],
                                         AF.Exp, scale=0.125)
                if t >= 0:
                    (c0, c1), (m0, m1) = MASK_APPLY[t]
                    nc.vector.tensor_mul(e[:, :, c0:c1], e[:, :, c0:c1],
                                         masks[:, t, :, m0:m1])
                nc.tensor.matmul(ctxA[:, cq:512], vA[:, blk, hA, :],
                                 e[:, 0, 0:w],
                                 start=(blk == 0), stop=(blk == nblk - 1))
                nc.tensor.matmul(ctxB[:, cq:512], vA[:, blk, hB, :],
                                 e[:, 1, 0:w],
                                 start=(blk == 0), stop=(blk == nblk - 1))

            def emit_norm(hp, j):
                """Normalize ctx^T out of PSUM into qT's freed j-tile."""
                ctxA, ctxB = ctxps.pop(hp)
                for ctxp, p0 in ((ctxA, 0), (ctxB, 64)):
                    denb = small_pool.tile([1, 512], F32, name="denb", bufs=1)
                    nc.vector.tensor_copy(denb[:], ctxp[64:65, :])
                    rec = small_pool.tile([1, 512], F32, name="rec", bufs=1)
                    nc.vector.reciprocal_approx_fast(rec[:], denb[:])
                    bcs = small_pool.tile([64, 512], F32, name="bcs")
                    nc.gpsimd.partition_broadcast(bcs[:], rec[:])
                    nc.vector.tensor_mul(qT[p0:p0 + 64, hp, ts(j, 512)],
                                         ctxp[0:64, :], bcs[:])

            def emit_A_group(j, g):
                """One PSUM-tile group of stage A(j): g=0..3 q/k pairs,
                g=4..5 v pairs."""
                xt = xts[j]
                if g < 4:
                    op, is_k = g // 2, g % 2
                    wt, bias, dstT = ((wk, bkt, kT) if is_k
                                      else (wq, bqt, qT))
                    pt = ps_mm.tile([128, 2, 512], F32, name="mm")
                    for half in range(2):
                        oc = 2 * op + half
                        for fc in range(NFC):
                            nc.tensor.matmul(pt[:, half, :],
                                             wt[:, fc, ts(oc, 128)], xt[:, fc],
                                             start=(fc == 0),
                                             stop=(fc == NFC - 1))
                        nc.scalar.activation(dstT[:, oc, ts(j, 512)],
                                             pt[:, half, :], AF.Identity,
                                             bias=bias[:, oc:oc + 1])
                else:
                    sp = g - 4
                    pv = ps_mm.tile([128, 2, 512], F32, name="mm")
                    for half in range(2):
                        sc = 2 * sp + half
                        for fc in range(NFC):
                            nc.tensor.matmul(pv[:, half, :],
                                             xt[:, fc, ts(sc, 128)], wv[:, fc],
                                             start=(fc == 0),
                                             stop=(fc == NFC - 1))
                        pv_r = pv[:, half, :].rearrange("p (h u) -> p h u",
                                                        u=HD)
                        nc.scalar.activation(vA[:, 4 * j + sc, :, 0:HD],
                                             pv_r, AF.Identity)

            def emit_C_group(sq):
                pp = ps_mm.tile([128, 2, 512], F32, name="mm")
                for oc in range(2):
                    for hc in range(NOC):
                        nc.tensor.matmul(pp[:, oc, :], qT[:, hc, ts(sq, 128)],
                                         wo[:, hc, ts(oc, 512)],
                                         start=(hc == 0), stop=(hc == NOC - 1))
                ot = poC.tile([128, 2, 512], F32, name="ot")
                nc.vector.tensor_copy(ot[:], pp[:])
                nc.sync.dma_start(
                    po_d[ts(sq, 128), :],
                    ot[:].rearrange("p a b -> p (a b)"))

            def emit_wave(j, fills):
                """Emit wave j's attention units with `fills` (stage A(j+1)
                and C group closures) woven between units, so the shared
                PSUM ring serves PE fill work in execution order and the
                PE never starves while ACT/DVE pace the exp."""
                units = []
                nblk = 4 * j + 4
                for hp in range(NHP):
                    for blk in range(nblk):
                        units.append((emit_blk, (hp, j, blk)))
                    units.append((emit_norm, (hp, j)))
                nf = len(fills)
                fi = 0
                for u, (fn, args) in enumerate(units):
                    fn(*args)
                    # after unit u, emit any fills scheduled by then
                    while fi < nf and (u + 1) * nf >= (fi + 1) * len(units):
                        fills[fi]()
                        fi += 1
                while fi < nf:
                    fills[fi]()
                    fi += 1

            # stage A(0) runs ahead of wave 0
            for g in range(6):
                emit_A_group(0, g)

            for j in range(NSQ):
                fills = []
                if j + 1 < NSQ:
                    xts[j + 1] = xin.tile([128, NFC, 512], BF16, name="xt")
                    for fc in range(NFC):
                        nc.sync.dma_start(xts[j + 1][:, fc],
                                          xT_d[ts(fc, 128), ts(j + 1, 512)])
                    fills += [(lambda jj=j + 1, gg=g: emit_A_group(jj, gg))
                              for g in range(6)]
                if j == 1:
                    for hc in range(NOC):
                        nc.sync.dma_start(wo[:, hc], wo_d[ts(hc, 128), :])
                # C groups for sq tiles whose ctx^T is complete
                cs = {0: [], 1: [0, 1, 2, 3], 2: [4, 5, 6, 7],
                      3: [8, 9, 10, 11]}[j]
                fills += [(lambda ss=sq: emit_C_group(ss)) for sq in cs]
                emit_wave(j, fills)

            for sq in range(12, NSK):
                emit_C_group(sq)


def make_masks():
    p = np.arange(128)[:, None]
    c = np.arange(512)[None, :]
    m = np.empty((128, 4, 512), dtype=np.float32)
    for t in range(4):
        m[:, t, :] = (c >= p + 128 * t).astype(np.float32)
    return np.ascontiguousarray(
        np.repeat(m[:, :, None, :], 2, axis=2).astype(ml_dtypes.bfloat16))


def _bf16(a):
    return np.ascontiguousarray(a.astype(ml_dtypes.bfloat16))


def make_in_maps(x, Wq, bq, Wk, bk, Wv, Wo):
    masks = make_masks()
    in_maps = []
    for c in range(NCORES):
        b, g = c // 2, c % 2
        sl = slice(g * DH, (g + 1) * DH)
        in_maps.append({
            "xT": _bf16(x[b].T),
            "wq": _bf16(Wq[:, sl]),
            "wk": _bf16(Wk[:, sl]),
            "wv": _bf16(Wv[:, sl]),
            "wo": _bf16(Wo[sl, :]),
            "bqt": np.ascontiguousarray(bq[sl].reshape(NOC, 128).T),
            "bkt": np.ascontiguousarray(bk[sl].reshape(NOC, 128).T),
            "masks": masks,
        })
    return in_maps


_CACHE = {}


def _get_program(dve_num=1, dve_den=3):
    key = ("prog", dve_num, dve_den)
    if key not in _CACHE:
        _CACHE[key] = build_program(dve_num=dve_num, dve_den=dve_den)
    return _CACHE[key]


def kernel(x, Wq, bq, Wk, bk, Wv, bv, Wo, bo, **run_kwargs):
    x = np.asarray(x, dtype=np.float32)
    Wq = np.asarray(Wq, dtype=np.float32)
    bq = np.asarray(bq, dtype=np.float32)
    Wk = np.asarray(Wk, dtype=np.float32)
    bk = np.asarray(bk, dtype=np.float32)
    Wv = np.asarray(Wv, dtype=np.float32)
    bv = np.asarray(bv, dtype=np.float32)
    Wo = np.asarray(Wo, dtype=np.float32)
    bo = np.asarray(bo, dtype=np.float32)

    run_kwargs.pop("f32r", None)
    dve_num = run_kwargs.pop("dve_num", 1)
    dve_den = run_kwargs.pop("dve_den", 3)
    nc = _get_program(dve_num=dve_num, dve_den=dve_den)
    in_maps = make_in_maps(x, Wq, bq, Wk, bk, Wv, Wo)
    res = run_bass_kernel_spmd(nc, in_maps, list(range(NCORES)), **run_kwargs)
    bo_fold = bv @ Wo + bo  # v-bias folded through the output projection
    out = np.empty((B, S, D), dtype=np.float32)
    for b in range(B):
        out[b] = res.results[2 * b]["po"] + res.results[2 * b + 1]["po"] + bo_fold
    _CACHE["last_results"] = res
    return out


# revision 8
# speedup vs baseline: 1.1680x; 1.1680x over previous
"""Multi-head masked self-attention on 8 TRN2 NeuronCores.

Problem: B=4, S=2048, D=1024, H=16 heads (hd=64), fp32.
  q,k,v = x@W* + b*; causal softmax(q k^T / 8) @ v; out = ctx @ Wo + bo.

Sharding: core c -> (batch b = c//2, head-group g = c%2 of 8 heads).
Each core computes a partial output projection over its 512 hidden dims;
the host sums the two partials per batch and adds bo (with bv@Wo folded in,
so no v-bias work on device).

On-device layout strategy (no on-device transposes needed):
  - host passes xT = x[b].T  [D, S] in bf16; all GEMM-side operands
    (x, Wq/Wk/Wv/Wo, q^T, k^T, ctx^T) are bf16: same PE throughput as
    f32r (1 col/cycle) but fast-weight-load halves LDWEIGHTS, and input
    DMA bytes halve
  - q^T, k^T computed directly as [512, S] (lhsT = W chunk, rhs = xT chunk)
  - v computed in natural [S, 512] layout, stored with a ones-column per
    head ("v_aug", f32r [S, 8*65]) so the ctx matmul accumulates the
    softmax denominator in PSUM row 64 for free
  - scores are computed transposed: st[sk, sq] = k q^T.  The two heads of
    a pair (2i, 2i+1) live in partitions 0:64 / 64:128 of the same kT/qT
    chunk, so their K=64 score matmuls land on PE row-groups (0,0) and
    (64,0) and run CONCURRENTLY (row tiling) when emitted back-to-back —
    2x on the score matmuls vs serial heads
  - exp via ACT (softmax without max-subtraction: scores are O(+-10));
    optionally a fraction of off-diagonal blocks run a one-instruction
    Schraudolph exp on DVE (i32 = A*s + B, bitcast to f32) to offload ACT
  - causal masking by multiplying only the triangular 128-col band of
    diagonal blocks with precomputed 0/1 masks (not the full block)
  - ctx^T[hd, sq] accumulated in PSUM = v_aug^T.T @ exp; normalization:
    fast approximate reciprocal of the den row straight out of PSUM,
    gpsimd partition_broadcast, multiply on DVE during evacuation
  - output projection uses ctx^T directly as lhsT (again no transpose);
    ctx^T aliases qT's storage (each qT j-tile dies as its wave completes)
  - QKV projections, attention waves, and the output projection are
    emitted so the Tile scheduler keeps the PE dense and warm
"""

import numpy as np
import ml_dtypes

import concourse.bass as bass
import concourse.mybir as mybir
import concourse.tile as tile
from concourse import bacc
from concourse.bass import ts
from concourse.bass_utils import run_bass_kernel_spmd

F32 = mybir.dt.float32
F32R = mybir.dt.float32r
BF16 = mybir.dt.bfloat16
I32 = mybir.dt.int32
I16 = mybir.dt.int16
AF = mybir.ActivationFunctionType
ALU = mybir.AluOpType

B, S, D, H, HD = 4, 2048, 1024, 16, 64
G = 2                 # head groups (cores per batch)
DH = D // G           # hidden dims per core = 512
HPC = H // G          # heads per core = 8
NHP = HPC // 2        # head pairs per core = 4
NCORES = 8

NSQ = S // 512        # 4 sq tiles of 512
NSK = S // 128        # 16 sk chunks of 128
NFC = D // 128        # 8 feature chunks
NOC = DH // 128       # 4 out-dim chunks of the per-core hidden

NWARM = 48            # HAM warmup matmuls while input DMAs land

# Schraudolph exp in bf16: exp(0.125*s) ~= bitcast_bf16(round(SCH_A*s + SCH_B)
# as int16); max rel err ~3.3%, mean +1.0% (fitted for s ~ N(0, 64))
SCH_A = float(np.float32(0.125 * (2 ** 7) / np.log(2)))
SCH_B = float(np.float32(127 * 128 - 5.5))

# per-diagonal-block (t = blk - 4j) live q-window: width and col offset.
# Each diag block only computes its live columns; the first 128 of them
# are the triangular boundary, masked by one shared (c >= p) triangle.
DIAG_W = {0: (512, 0), 1: (384, 128), 2: (256, 256), 3: (128, 384)}


def build_program(dve_num=0, dve_den=2):
    """Build the single-core SPMD Bass program (same program on all 8 cores).

    Off-diagonal exp blocks with (counter % dve_den) < dve_num run on DVE
    via Schraudolph; the rest (and all diagonal blocks) run exact exp on ACT.
    """
    nc = bacc.Bacc("TRN2", target_bir_lowering=False, debug=False)

    xT_d = nc.dram_tensor("xT", [D, S], BF16, kind="ExternalInput").ap()
    wq_d = nc.dram_tensor("wq", [D, DH], BF16, kind="ExternalInput").ap()
    wk_d = nc.dram_tensor("wk", [D, DH], BF16, kind="ExternalInput").ap()
    wv_d = nc.dram_tensor("wv", [D, DH], BF16, kind="ExternalInput").ap()
    wo_d = nc.dram_tensor("wo", [DH, D], BF16, kind="ExternalInput").ap()
    bqt_d = nc.dram_tensor("bqt", [128, NOC], F32, kind="ExternalInput").ap()
    bkt_d = nc.dram_tensor("bkt", [128, NOC], F32, kind="ExternalInput").ap()
    mask_d = nc.dram_tensor("masks", [128, 2, 128], BF16,
                            kind="ExternalInput").ap()
    po_d = nc.dram_tensor("po", [S, D], F32, kind="ExternalOutput").ap()

    with tile.TileContext(nc) as tc:
        _emit(tc, xT_d, wq_d, wk_d, wv_d, wo_d, bqt_d, bkt_d, mask_d, po_d,
              dve_num, dve_den)
    nc.compile()
    return nc


def _emit(tc, xT_d, wq_d, wk_d, wv_d, wo_d, bqt_d, bkt_d, mask_d, po_d,
          dve_num, dve_den):
    nc = tc.nc
    PS = bass.MemorySpace.PSUM

    with (
        tc.tile_pool(name="persist", bufs=1) as persist,
        tc.tile_pool(name="qkv", bufs=1) as qkv_pool,
        tc.tile_pool(name="exp", bufs=6) as exp_pool,
        tc.tile_pool(name="small", bufs=2) as small_pool,
        tc.tile_pool(name="ps_st", bufs=2, space=PS) as ps_st,
        tc.tile_pool(name="ps_mm", bufs=1, space=PS) as ps_mm,
        tc.tile_pool(name="ps_ctx", bufs=2, space=PS) as ps_ctx,
    ):
        bqt = persist.tile([128, NOC], F32)
        bkt = persist.tile([128, NOC], F32)
        nc.sync.dma_start(bqt[:], bqt_d[:])
        nc.sync.dma_start(bkt[:], bkt_d[:])

        # HAM pre-warm: throwaway matmuls on zeros while input DMAs land,
        # so the PE clock is at 2.4 GHz when real work starts
        zw = persist.tile([128, 512], BF16)
        nc.vector.memset(zw[:], 0.0)
        pwarm = ps_mm.tile([128, 2, 512], F32, name="mm")
        for i in range(NWARM):
            nc.tensor.matmul(pwarm[:, i % 2, :], zw[:, 0:128], zw[:],
                             start=True, stop=True)

        # causal triangle mask (c >= p), duplicated across the head-pair
        # dim; after per-t width trimming every diagonal block multiplies
        # its first 128 cols by this same [128, 128] triangle
        masks = persist.tile([128, 2, 128], BF16)
        nc.sync.dma_start(masks[:], mask_d[:])

        # persistent activations.  qT doubles as ctx^T storage: wave j's
        # evacuation overwrites qT[:, :, j-tile] right after the last
        # score matmul that reads it (disjoint partition rows per head).
        qT = qkv_pool.tile([128, NOC, S], BF16)       # q^T + bq, then ctx^T
        kT = qkv_pool.tile([128, NOC, S], BF16)       # k^T + bk   [512, S]
        vA = qkv_pool.tile([128, NSK, HPC, HD + 1], BF16)  # v + ones col
        nc.vector.memset(vA[:, :, :, HD:HD + 1], 1.0)

        exp_ctr = [0]

        with (
            tc.tile_pool(name="wtsA", bufs=1) as wtsA,
            tc.tile_pool(name="xin", bufs=2) as xin,
            tc.tile_pool(name="woC", bufs=1) as woC,
            tc.tile_pool(name="poC", bufs=3) as poC,
        ):
            xts = {}
            xts[0] = xin.tile([128, NFC, 512], BF16, name="xt")
            wq = wtsA.tile([128, NFC, DH], BF16)
            wk = wtsA.tile([128, NFC, DH], BF16)
            wv = wtsA.tile([128, NFC, DH], BF16)
            wo = woC.tile([128, NOC, D], BF16)
            # interleave x/wq chunks so the first K-chain starts ASAP
            for fc in range(NFC):
                nc.sync.dma_start(xts[0][:, fc], xT_d[ts(fc, 128), ts(0, 512)])
                nc.sync.dma_start(wq[:, fc], wq_d[ts(fc, 128), :])
            for fc in range(NFC):
                nc.sync.dma_start(wk[:, fc], wk_d[ts(fc, 128), :])
            for fc in range(NFC):
                nc.sync.dma_start(wv[:, fc], wv_d[ts(fc, 128), :])

            # --- attention wave pieces (emitted per unit so fill work can
            # be woven between units) -----------------------------------
            ctxps = {}

            def emit_blk(hp, j, blk):
                """One (head-pair, sq-tile, k-block) unit: the two heads'
                K=64 score matmuls run concurrently on PE row-groups 0/64."""
                nblk = 4 * j + 4
                hA, hB = 2 * hp, 2 * hp + 1
                if blk == 0:
                    ctxps[hp] = (ps_ctx.tile([65, 512], F32, name="ctxp"),
                                 ps_ctx.tile([65, 512], F32, name="ctxp"))
                ctxA, ctxB = ctxps[hp]
                t = blk - 4 * j  # >= 0 on diagonal blocks
                w, cq = DIAG_W[t] if t >= 0 else (512, 0)
                stp = ps_st.tile([128, 2, 512], F32, name="st")
                nc.tensor.matmul(stp[:, 0, 0:w], kT[0:64, hp, ts(blk, 128)],
                                 qT[0:64, hp, ts(j, 512)][:, cq:512],
                                 start=True, stop=True)
                nc.tensor.matmul(stp[:, 1, 0:w], kT[64:128, hp, ts(blk, 128)],
                                 qT[64:128, hp, ts(j, 512)][:, cq:512],
                                 start=True, stop=True)
                e = exp_pool.tile([128, 2, 512], BF16, name="e")
                use_dve = (t < 0 and (exp_ctr[0] % dve_den) < dve_num)
                if t < 0:
                    exp_ctr[0] += 1
                if use_dve:
                    nc.vector.tensor_scalar(
                        e[:, :, 0:w].bitcast(I16), stp[:, :, 0:w],
                        SCH_A, SCH_B, ALU.mult, ALU.add)
                else:
                    nc.scalar.activation(e[:, :, 0:w], stp[:, :, 0:w],
                                         AF.Exp, scale=0.125)
                if t >= 0:
                    nc.vector.tensor_mul(e[:, :, 0:128], e[:, :, 0:128],
                                         masks[:, :, :])
                nc.tensor.matmul(ctxA[:, cq:512], vA[:, blk, hA, :],
                                 e[:, 0, 0:w],
                                 start=(blk == 0), stop=(blk == nblk - 1))
                nc.tensor.matmul(ctxB[:, cq:512], vA[:, blk, hB, :],
                                 e[:, 1, 0:w],
                                 start=(blk == 0), stop=(blk == nblk - 1))

            def emit_norm(hp, j):
                """Normalize ctx^T out of PSUM into qT's freed j-tile."""
                ctxA, ctxB = ctxps.pop(hp)
                for ctxp, p0 in ((ctxA, 0), (ctxB, 64)):
                    denb = small_pool.tile([1, 512], F32, name="denb", bufs=1)
                    nc.vector.tensor_copy(denb[:], ctxp[64:65, :])
                    rec = small_pool.tile([1, 512], F32, name="rec", bufs=1)
                    nc.vector.reciprocal_approx_fast(rec[:], denb[:])
                    bcs = small_pool.tile([64, 512], F32, name="bcs")
                    nc.gpsimd.partition_broadcast(bcs[:], rec[:])
                    nc.vector.tensor_mul(qT[p0:p0 + 64, hp, ts(j, 512)],
                                         ctxp[0:64, :], bcs[:])

            def emit_A_group(j, g):
                """One PSUM-tile group of stage A(j): g=0..3 q/k pairs,
                g=4..5 v pairs."""
                xt = xts[j]
                if g < 4:
                    op, is_k = g // 2, g % 2
                    wt, bias, dstT = ((wk, bkt, kT) if is_k
                                      else (wq, bqt, qT))
                    pt = ps_mm.tile([128, 2, 512], F32, name="mm")
                    for half in range(2):
                        oc = 2 * op + half
                        for fc in range(NFC):
                            nc.tensor.matmul(pt[:, half, :],
                                             wt[:, fc, ts(oc, 128)], xt[:, fc],
                                             start=(fc == 0),
                                             stop=(fc == NFC - 1))
                        nc.vector.tensor_scalar_add(dstT[:, oc, ts(j, 512)],
                                                    pt[:, half, :],
                                                    bias[:, oc:oc + 1])
                else:
                    sp = g - 4
                    pv = ps_mm.tile([128, 2, 512], F32, name="mm")
                    for half in range(2):
                        sc = 2 * sp + half
                        for fc in range(NFC):
                            nc.tensor.matmul(pv[:, half, :],
                                             xt[:, fc, ts(sc, 128)], wv[:, fc],
                                             start=(fc == 0),
                                             stop=(fc == NFC - 1))
                        pv_r = pv[:, half, :].rearrange("p (h u) -> p h u",
                                                        u=HD)
                        nc.vector.tensor_copy(vA[:, 4 * j + sc, :, 0:HD],
                                               pv_r)

            def emit_C_group(sq):
                pp = ps_mm.tile([128, 2, 512], F32, name="mm")
                for oc in range(2):
                    for hc in range(NOC):
                        nc.tensor.matmul(pp[:, oc, :], qT[:, hc, ts(sq, 128)],
                                         wo[:, hc, ts(oc, 512)],
                                         start=(hc == 0), stop=(hc == NOC - 1))
                ot = poC.tile([128, 2, 512], F32, name="ot")
                nc.vector.tensor_copy(ot[:], pp[:])
                nc.sync.dma_start(
                    po_d[ts(sq, 128), :],
                    ot[:].rearrange("p a b -> p (a b)"))

            def emit_wave(j, fills):
                """Emit wave j's attention units with `fills` (stage A(j+1)
                and C group closures) woven between units, so the shared
                PSUM ring serves PE fill work in execution order and the
                PE never starves while ACT/DVE pace the exp."""
                units = []
                nblk = 4 * j + 4
                for hp in range(NHP):
                    for blk in range(nblk):
                        units.append((emit_blk, (hp, j, blk)))
                    units.append((emit_norm, (hp, j)))
                nf = len(fills)
                fi = 0
                for u, (fn, args) in enumerate(units):
                    fn(*args)
                    # after unit u, emit any fills scheduled by then
                    while fi < nf and (u + 1) * nf >= (fi + 1) * len(units):
                        fills[fi]()
                        fi += 1
                while fi < nf:
                    fills[fi]()
                    fi += 1

            # stage A(0) runs ahead of wave 0
            for g in range(6):
                emit_A_group(0, g)

            for j in range(NSQ):
                fills = []
                if j + 1 < NSQ:
                    xts[j + 1] = xin.tile([128, NFC, 512], BF16, name="xt")
                    for fc in range(NFC):
                        nc.sync.dma_start(xts[j + 1][:, fc],
                                          xT_d[ts(fc, 128), ts(j + 1, 512)])
                    fills += [(lambda jj=j + 1, gg=g: emit_A_group(jj, gg))
                              for g in range(6)]
                if j == 1:
                    for hc in range(NOC):
                        nc.sync.dma_start(wo[:, hc], wo_d[ts(hc, 128), :])
                # C groups for sq tiles whose ctx^T is complete
                cs = {0: [], 1: [0, 1, 2, 3], 2: [4, 5, 6, 7],
                      3: [8, 9, 10, 11]}[j]
                fills += [(lambda ss=sq: emit_C_group(ss)) for sq in cs]
                emit_wave(j, fills)

            for sq in range(12, NSK):
                emit_C_group(sq)


def make_masks():
    p = np.arange(128)[:, None]
    c = np.arange(128)[None, :]
    m = (c >= p).astype(np.float32)
    return np.ascontiguousarray(
        np.repeat(m[:, None, :], 2, axis=1).astype(ml_dtypes.bfloat16))


def _bf16(a):
    return np.ascontiguousarray(a.astype(ml_dtypes.bfloat16))


def make_in_maps(x, Wq, bq, Wk, bk, Wv, Wo):
    masks = make_masks()
    in_maps = []
    for c in range(NCORES):
        b, g = c // 2, c % 2
        sl = slice(g * DH, (g + 1) * DH)
        in_maps.append({
            "xT": _bf16(x[b].T),
            "wq": _bf16(Wq[:, sl]),
            "wk": _bf16(Wk[:, sl]),
            "wv": _bf16(Wv[:, sl]),
            "wo": _bf16(Wo[sl, :]),
            "bqt": np.ascontiguousarray(bq[sl].reshape(NOC, 128).T),
            "bkt": np.ascontiguousarray(bk[sl].reshape(NOC, 128).T),
            "masks": masks,
        })
    return in_maps


_CACHE = {}


def _get_program(dve_num=0, dve_den=3):
    key = ("prog", dve_num, dve_den)
    if key not in _CACHE:
        _CACHE[key] = build_program(dve_num=dve_num, dve_den=dve_den)
    return _CACHE[key]


def kernel(x, Wq, bq, Wk, bk, Wv, bv, Wo, bo, **run_kwargs):
    x = np.asarray(x, dtype=np.float32)
    Wq = np.asarray(Wq, dtype=np.float32)
    bq = np.asarray(bq, dtype=np.float32)
    Wk = np.asarray(Wk, dtype=np.float32)
    bk = np.asarray(bk, dtype=np.float32)
    Wv = np.asarray(Wv, dtype=np.float32)
    bv = np.asarray(bv, dtype=np.float32)
    Wo = np.asarray(Wo, dtype=np.float32)
    bo = np.asarray(bo, dtype=np.float32)

    run_kwargs.pop("f32r", None)
    dve_num = run_kwargs.pop("dve_num", 0)
    dve_den = run_kwargs.pop("dve_den", 3)
    nc = _get_program(dve_num=dve_num, dve_den=dve_den)
    in_maps = make_in_maps(x, Wq, bq, Wk, bk, Wv, Wo)
    res = run_bass_kernel_spmd(nc, in_maps, list(range(NCORES)), **run_kwargs)
    bo_fold = bv @ Wo + bo  # v-bias folded through the output projection
    out = np.empty((B, S, D), dtype=np.float32)
    for b in range(B):
        out[b] = res.results[2 * b]["po"] + res.results[2 * b + 1]["po"] + bo_fold
    _CACHE["last_results"] = res
    return out
